# revision 1
# baseline (speedup 1.0000x reference)
"""Multi-head cross attention on 8 trn2 NeuronCores.

Problem: B=2, T=4096, EMB=512, H=8 heads (head dim 64), fp32 I/O.
  q = x1 @ Wq.T + bq ; k,v from x2 ; S = q k^T / sqrt(512) ;
  softmax over keys with -1e10 masking ; out = (A v) @ Wu.T + bu.

Sharding: core c handles batch b = c//4 and query rows
[1024*(c%4), 1024*(c%4+1)).  Each core computes K,V for its batch in
full (4-way duplication), its own Q chunk, attention, and out-proj.

Device-side layout choices:
  - All matmul operands fp16 (PE rate is dtype-independent; fp16 halves
    DMA/SBUF and keeps ~1e-3 accuracy), accumulation fp32 in PSUM.
  - Scores computed TRANSPOSED, S^T[key, query]: contraction over the
    head dim requires Q^T/K^T (head-dim on partitions), which fall out
    of computing the projections transposed from x^T inputs (host
    pre-transposes x1/x2/W).  With keys on partitions, P^T = exp(S^T)*M^T
    feeds the AV matmul directly as its stationary-side contraction
    without any on-chip transposes.
  - Scores are small (|S| < ~1) so exp needs no max-subtraction; the
    1/sqrt(512) scale is folded into the ACT exp instruction.
  - V is stored interleaved [key, head, 65] with a ones column so the
    AV matmul also produces the softmax denominators r[q] (row 64).
  - Normalization is deferred: Y^T_h / r_h via reciprocal + a K=1
    broadcast matmul + one DVE multiply per (head, chunk).
  - 2 heads are packed per scores pass via tile_position row-tiling
    (contraction=64 -> rows 0-63 / 64-127 run concurrently).
"""
import math
import os
from contextlib import ExitStack

import numpy as np

import concourse.bass as bass
import concourse.bacc as bacc
import concourse.tile as tile
import concourse.mybir as mybir
from concourse.bass_utils import run_bass_kernel_spmd

F16 = mybir.dt.float16
F32 = mybir.dt.float32
EXP = mybir.ActivationFunctionType.Exp

EMB, H, D, CT = 512, 8, 64, 4  # emb, heads, head dim, emb/128

FULL_CFG = dict(T=4096, QC=1024)  # keys per batch, query rows per core
MINI_CFG = dict(T=512, QC=256)


def attention_body(ctx, tc, io, cfg):
    nc = tc.nc
    T, QC = cfg["T"], cfg["QC"]
    KT = T // 128            # key tiles
    NG = KT // 2             # key-tile groups of 2
    CH = min(512, QC)        # query chunk width
    NCH = QC // CH
    scale = 1.0 / math.sqrt(EMB)

    pw = ctx.enter_context(tc.tile_pool(name="w", bufs=1))
    pk = ctx.enter_context(tc.tile_pool(name="kt", bufs=1))
    pv = ctx.enter_context(tc.tile_pool(name="v", bufs=1))
    pq = ctx.enter_context(tc.tile_pool(name="qt", bufs=1))

    # persistent weights / biases / constants
    wq = [pw.tile([128, EMB], F16, tag=f"wq{i}", name=f"wq{i}") for i in range(CT)]
    wk = [pw.tile([128, EMB], F16, tag=f"wk{i}", name=f"wk{i}") for i in range(CT)]
    wv = [pw.tile([128, EMB], F16, tag=f"wv{i}", name=f"wv{i}") for i in range(CT)]
    wu = [pw.tile([128, EMB], F16, tag=f"wu{i}", name=f"wu{i}") for i in range(CT)]
    for i in range(CT):
        nc.sync.dma_start(wq[i][:], io["wqT"][bass.ts(i, 128), :])
        nc.sync.dma_start(wk[i][:], io["wkT"][bass.ts(i, 128), :])
        nc.sync.dma_start(wv[i][:], io["wvT"][bass.ts(i, 128), :])
        nc.sync.dma_start(wu[i][:], io["wuT"][bass.ts(i, 128), :])
    bqr = pw.tile([128, CT], F32, tag="bqr", name="bqr")
    bkr = pw.tile([128, CT], F32, tag="bkr", name="bkr")
    bvb = pw.tile([128, EMB], F32, tag="bvb", name="bvb")
    bub = pw.tile([128, EMB], F32, tag="bub", name="bub")
    nc.sync.dma_start(bqr[:], io["bqr"][:, :])
    nc.sync.dma_start(bkr[:], io["bkr"][:, :])
    nc.sync.dma_start(bvb[:], io["bvb"][:, :])
    nc.sync.dma_start(bub[:], io["bub"][:, :])
    ones = pw.tile([1, D], F16, tag="ones", name="ones")
    nc.vector.memset(ones[:], 1.0)

    # persistent K^T [emb, T], V [T, head, 65(+pad)], Q^T [emb, QC]
    kt = [pk.tile([128, T], F16, tag=f"kt{i}", name=f"kt{i}") for i in range(CT)]
    v = pv.tile([128, KT, H, 66], F16, tag="v", name="v")
    nc.vector.memset(v[:, :, :, 64:65], 1.0)
    qt = [pq.tile([128, QC], F16, tag=f"qt{i}", name=f"qt{i}") for i in range(CT)]

    with tc.tile_pool(name="x", bufs=1) as px, \
         tc.tile_pool(name="pp", bufs=1, space="PSUM") as pp:
        x2t = [px.tile([128, T], F16, tag=f"x2t{i}", name=f"x2t{i}") for i in range(CT)]
        x1t = [px.tile([128, QC], F16, tag=f"x1t{i}", name=f"x1t{i}") for i in range(CT)]
        for i in range(CT):
            nc.sync.dma_start(x1t[i][:], io["x1T"][bass.ts(i, 128), :])
            for hf in range(2):
                nc.sync.dma_start(x2t[i][:, bass.ts(hf, T // 2)],
                                  io["x2T"][bass.ts(i, 128), bass.ts(hf, T // 2)])

        # Q^T[e,q] = sum_c WqT[c,e] * x1T[c,q]  (+ bq per-partition)
        for e in range(CT):
            for t in range(QC // CH):
                ps = pp.tile([128, CH], F32, tag=f"ps{t}", name="ps2")
                for c in range(CT):
                    nc.tensor.matmul(ps[:], wq[c][:, bass.ts(e, 128)],
                                     x1t[c][:, bass.ts(t, CH)],
                                     start=(c == 0), stop=(c == CT - 1))
                nc.vector.tensor_scalar_add(qt[e][:, bass.ts(t, CH)], ps[:],
                                            bqr[:, e:e + 1])
        # K^T[e,t] = sum_c WkT[c,e] * x2T[c,t] (+ bk); V[t,e] interleaved.
        # Stationary (wk) reused across 8 t-chunks per LDWEIGHTS.
        NT8 = min(8, T // 512)
        for e in range(CT):
            for tb in range(T // 512 // NT8):
                pss = [pp.tile([128, 512], F32, tag=f"ps{j}", name=f"ps{j}")
                       for j in range(NT8)]
                for c in range(CT):
                    for j in range(NT8):
                        nc.tensor.matmul(pss[j][:], wk[c][:, bass.ts(e, 128)],
                                         x2t[c][:, bass.ts(tb * NT8 + j, 512)],
                                         start=(c == 0), stop=(c == CT - 1))
                for j in range(NT8):
                    nc.vector.tensor_scalar_add(
                        kt[e][:, bass.ts(tb * NT8 + j, 512)], pss[j][:],
                        bkr[:, e:e + 1])
            if e > 0:
                continue
            # V right after K^T e-tile 0 so attention pair 0 can start
            for t in range(KT):
                ps = pp.tile([128, EMB], F32, tag="ps0", name="psv")
                for c in range(CT):
                    nc.tensor.matmul(ps[:], x2t[c][:, bass.ts(t, 128)], wv[c][:],
                                     start=(c == 0), stop=(c == CT - 1))
                nc.vector.tensor_add(
                    v[:, t, :, 0:64],
                    ps[:].rearrange("p (h d) -> p h d", h=H),
                    bvb[:].rearrange("p (h d) -> p h d", h=H))

    # optional debug dumps of intermediates
    if "dbg" in io:
        for e in range(CT):
            nc.sync.dma_start(io["dbg_qt"][bass.ts(e, 128), :], qt[e][:])
            nc.sync.dma_start(io["dbg_kt"][bass.ts(e, 128), :], kt[e][:])
        for t in range(KT):
            nc.sync.dma_start(
                io["dbg_v"][:, :].rearrange("p (a b) -> p a b", a=KT)[:, t, :],
                v[:, t, :, :].rearrange("p a b -> p (a b)"))

    # attention — both query chunks processed together so every matmul
    # stationary (kt slices, V tiles) is loaded once and reused, and exp
    # covers a full 4-bank PSUM span per key tile.
    QW = min(512, QC)        # matmul moving width (PSUM bank limit)
    NB = QC // QW            # query blocks
    with tc.tile_pool(name="ps_s", bufs=1, space="PSUM") as ps_s, \
         tc.tile_pool(name="ps_av", bufs=2, space="PSUM") as ps_av, \
         tc.tile_pool(name="pe", bufs=2) as pe, \
         tc.tile_pool(name="ppp", bufs=2) as ppp, \
         tc.tile_pool(name="pm", bufs=min(KT, 16)) as pm, \
         tc.tile_pool(name="py", bufs=CT) as py, \
         tc.tile_pool(name="pys", bufs=4) as pys, \
         tc.tile_pool(name="prr", bufs=2) as prr, \
         tc.tile_pool(name="po", bufs=2) as po:
        yts = [py.tile([128, QC], F16, tag="yt", name=f"yt{e}")
               for e in range(CT)]
        for pr in range(CT):  # head pair
            av = [ps_av.tile([65, QC], F32, tag="av", name="av")
                  for _ in range(2)]

            def mul_and_av(kk, e16, pt, mkt):
                nc.vector.tensor_mul(
                    pt[:].rearrange("p (c h q) -> p c h q", c=NB, h=2),
                    e16[:].rearrange("p (c h q) -> p c h q", c=NB, h=2),
                    mkt[:].rearrange("p (c q) -> p c q", c=NB)
                    .unsqueeze(2).broadcast_to([128, NB, 2, QW]))
                for hh in range(2):
                    for cb in range(NB):
                        nc.tensor.matmul(
                            av[hh][:, bass.ts(cb, QW)],
                            v[:, kk, 2 * pr + hh, 0:65],
                            pt[:, bass.ds((2 * cb + hh) * QW, QW)],
                            start=(kk == 0), stop=(kk == KT - 1))

            prev = None
            for kk in range(KT):
                mkt = pm.tile([128, QC], F16, tag="mk", name="mk")
                nc.sync.dma_start(mkt[:], io["maskT"][bass.ts(kk, 128), :])
                e16 = pe.tile([128, 2 * QC], F16, tag="E", name="e16")
                pt = ppp.tile([128, 2 * QC], F16, tag="P", name="pt")
                ps = ps_s.tile([128, 2 * QC], F32, tag="s", name="ps_s")
                for cb in range(NB):
                    for hh in range(2):  # head within pair
                        nc.tensor.matmul(
                            ps[:, bass.ds((2 * cb + hh) * QW, QW)],
                            kt[pr][bass.ds(64 * hh, 64), bass.ts(kk, 128)],
                            qt[pr][bass.ds(64 * hh, 64), bass.ts(cb, QW)],
                            start=True, stop=True,
                            tile_position=(64 * hh, 0))
                nc.scalar.activation(e16[:], ps[:], EXP, scale=scale)
                if prev is not None:
                    mul_and_av(*prev)
                prev = (kk, e16, pt, mkt)
            mul_and_av(*prev)

            # normalize: Y^T_h / r_h, r from the ones column (row 64)
            for hh in range(2):
                ysb = pys.tile([65, QC], F32, tag="ys", name="ysb")
                nc.scalar.copy(ysb[:], av[hh][:])
                r0 = prr.tile([1, QC], F32, tag="r0", name="r0")
                nc.vector.tensor_copy(r0[:], ysb[64:65, :])
                rr32 = prr.tile([1, QC], F32, tag="rr32", name="rr32")
                nc.vector.reciprocal_approx_fast(rr32[:], r0[:])
                rr = prr.tile([1, QC], F16, tag="rr", name="rr")
                with nc.allow_low_precision(reason="fp16 recip copy ok"):
                    nc.vector.tensor_copy(rr[:], rr32[:])
                bc = ps_av.tile([64, QC], F32, tag="av", name="bc")
                for cb in range(NB):
                    nc.tensor.matmul(bc[:, bass.ts(cb, QW)], ones[:],
                                     rr[:, bass.ts(cb, QW)],
                                     start=True, stop=True)
                nc.vector.tensor_mul(yts[pr][bass.ds(64 * hh, 64), :],
                                     ysb[0:64, :], bc[:])
        # out[q, :] = sum_e Y^T[e, q] * WuT[e, :] + bu
        for qi in range(QC // 128):
            pso = ps_av.tile([128, EMB], F32, tag="av", name="pso")
            for e in range(CT):
                nc.tensor.matmul(pso[:], yts[e][:, bass.ts(qi, 128)], wu[e][:],
                                 start=(e == 0), stop=(e == CT - 1))
            osb = po.tile([128, EMB], F32, tag="o", name="osb")
            nc.vector.tensor_add(osb[:], pso[:], bub[:])
            nc.sync.dma_start(io["out"][bass.ts(qi, 128), :], osb[:])


def build(cfg, num_devices=8, dbg=False):
    T, QC = cfg["T"], cfg["QC"]
    nc = bacc.Bacc("TRN2", target_bir_lowering=False, debug=False,
                   num_devices=num_devices)
    io = {
        "x1T": nc.dram_tensor("x1T", [EMB, QC], F16, kind="ExternalInput").ap(),
        "x2T": nc.dram_tensor("x2T", [EMB, T], F16, kind="ExternalInput").ap(),
        "maskT": nc.dram_tensor("maskT", [T, QC], F16, kind="ExternalInput").ap(),
        "wqT": nc.dram_tensor("wqT", [EMB, EMB], F16, kind="ExternalInput").ap(),
        "wkT": nc.dram_tensor("wkT", [EMB, EMB], F16, kind="ExternalInput").ap(),
        "wvT": nc.dram_tensor("wvT", [EMB, EMB], F16, kind="ExternalInput").ap(),
        "wuT": nc.dram_tensor("wuT", [EMB, EMB], F16, kind="ExternalInput").ap(),
        "bqr": nc.dram_tensor("bqr", [128, CT], F32, kind="ExternalInput").ap(),
        "bkr": nc.dram_tensor("bkr", [128, CT], F32, kind="ExternalInput").ap(),
        "bvb": nc.dram_tensor("bvb", [128, EMB], F32, kind="ExternalInput").ap(),
        "bub": nc.dram_tensor("bub", [128, EMB], F32, kind="ExternalInput").ap(),
        "out": nc.dram_tensor("out", [QC, EMB], F32, kind="ExternalOutput").ap(),
    }
    if dbg:
        io["dbg"] = True
        CH = min(512, QC)
        io["dbg_qt"] = nc.dram_tensor("dbg_qt", [EMB, QC], F16, kind="ExternalOutput").ap()
        io["dbg_kt"] = nc.dram_tensor("dbg_kt", [EMB, T], F16, kind="ExternalOutput").ap()
        io["dbg_v"] = nc.dram_tensor("dbg_v", [128, (T // 128) * H * 66], F16, kind="ExternalOutput").ap()
        io["dbg_e"] = nc.dram_tensor("dbg_e", [128, 2 * CH], F16, kind="ExternalOutput").ap()
        io["dbg_p"] = nc.dram_tensor("dbg_p", [128, 2 * CH], F16, kind="ExternalOutput").ap()
        io["dbg_y"] = nc.dram_tensor("dbg_y", [65, CH], F32, kind="ExternalOutput").ap()
    with tile.TileContext(nc) as tc:
        with ExitStack() as ctx:
            attention_body(ctx, tc, io, cfg)
    nc.compile()
    return nc


def host_prep(x1, x2, mask, Wq, bq, Wk, bk, Wv, bv, Wu, bu, cfg):
    """Build the 8 per-core input maps from full inputs."""
    T, QC = cfg["T"], cfg["QC"]
    shared = {
        "wqT": np.ascontiguousarray(Wq.T).astype(np.float16),
        "wkT": np.ascontiguousarray(Wk.T).astype(np.float16),
        "wvT": np.ascontiguousarray(Wv.T).astype(np.float16),
        "wuT": np.ascontiguousarray(Wu.T).astype(np.float16),
        "bqr": np.ascontiguousarray(bq.reshape(CT, 128).T).astype(np.float32),
        "bkr": np.ascontiguousarray(bk.reshape(CT, 128).T).astype(np.float32),
        "bvb": np.ascontiguousarray(np.broadcast_to(bv, (128, EMB))).astype(np.float32),
        "bub": np.ascontiguousarray(np.broadcast_to(bu, (128, EMB))).astype(np.float32),
    }
    x2T = [x2[b].T.astype(np.float16) for b in range(x1.shape[0])]
    in_maps = []
    n_cores = (x1.shape[0] * x1.shape[1]) // QC
    per_b = x1.shape[1] // QC
    for c in range(n_cores):
        b, q0 = c // per_b, (c % per_b) * QC
        in_maps.append(dict(
            shared,
            x1T=x1[b, q0:q0 + QC, :].T.astype(np.float16),
            x2T=x2T[b],
            maskT=mask[b, q0:q0 + QC, :].T.astype(np.float16),
        ))
    return in_maps


_NC_CACHE = {}


def kernel(x1, x2, mask, Wq, bq, Wk, bk, Wv, bv, Wu, bu):
    cfg = FULL_CFG
    B, TQ, _ = x1.shape
    in_maps = host_prep(np.asarray(x1, np.float32), np.asarray(x2, np.float32),
                        np.asarray(mask), np.asarray(Wq, np.float32),
                        np.asarray(bq, np.float32), np.asarray(Wk, np.float32),
                        np.asarray(bk, np.float32), np.asarray(Wv, np.float32),
                        np.asarray(bv, np.float32), np.asarray(Wu, np.float32),
                        np.asarray(bu, np.float32), cfg)
    key = (cfg["T"], cfg["QC"])
    if key not in _NC_CACHE:
        _NC_CACHE[key] = build(cfg)
    nc = _NC_CACHE[key]
    res = run_bass_kernel_spmd(nc, in_maps, core_ids=list(range(8)),
                               trace=bool(os.environ.get("KERNEL_TRACE")))
    if os.environ.get("KERNEL_TRACE"):
        kernel.last_exec_ns = res.exec_time_ns
        kernel.last_results = res
    out = np.empty((B, TQ, EMB), np.float32)
    per_b = TQ // cfg["QC"]
    for c in range(8):
        b, q0 = c // per_b, (c % per_b) * cfg["QC"]
        out[b, q0:q0 + cfg["QC"], :] = res.results[c]["out"]
    return out



# revision 10
# speedup vs baseline: 1.2136x; 1.2136x over previous
"""Multi-head cross attention on 8 trn2 NeuronCores.

Problem: B=2, T=4096, EMB=512, H=8 heads (head dim 64), fp32 I/O.
  q = x1 @ Wq.T + bq ; k,v from x2 ; S = q k^T / sqrt(512) ;
  softmax over keys with -1e10 masking ; out = (A v) @ Wu.T + bu.

Sharding: core c handles batch b = c//4 and query rows
[1024*(c%4), 1024*(c%4+1)).  Each core computes K,V for its batch in
full (4-way duplication), its own Q chunk, attention, and out-proj.

Device-side design (v2) — the kernel is ACT(exp)-bound, so everything
is organized to keep the Activation engine streaming EXPs gaplessly:
  - Scores computed TRANSPOSED, S^T[key, query] (head dim on PE
    partitions), from host-pre-transposed fp16 inputs.  2 heads packed
    per scores pass via tile_position row-tiling.
  - Queries processed in two 512-wide halves so each (head-pair, kk)
    score tile is [128, 1024] = 2 PSUM banks; tag-rotated double
    buffering (4 banks) lets EXP(kk+1) run while scores(kk+2) fill.
  - AV accumulates per head in [65, QW] PSUM (ones column in V gives
    softmax denominators); 2 heads x 1 bank, rotated (2 banks).
  - K projection for e-tiles 1-3 trickles through a single [128,1024]
    PSUM tag (2 banks) DURING attention (DVE bias adds); e-tile 0, Q
    and V are computed up front with ACT/GpSimd bias adds (ACT is idle
    pre-attention; Identity and Exp share an activation table so no
    table reloads).  x2^T is DMA'd column-block-first so K/V proj
    start before the transfer completes.
  - Normalization: r rows copied out via GpSimd, reciprocal on DVE,
    K=1 ones-matmul broadcast, final scale on DVE; the broadcast tile
    reuses the K-proj PSUM tag, out-proj PSUM reuses the av tag, so
    total PSUM = 4+2+2 = 8 banks with no pool barriers inside the
    attention phase.
"""
import math
import os
from contextlib import ExitStack

import numpy as np

import concourse.bass as bass
import concourse.bacc as bacc
import concourse.tile as tile
import concourse.mybir as mybir
from concourse.bass_utils import run_bass_kernel_spmd

F16 = mybir.dt.float16
F32 = mybir.dt.float32
EXP = mybir.ActivationFunctionType.Exp

EMB, H, D, CT = 512, 8, 64, 4  # emb, heads, head dim, emb/128

FULL_CFG = dict(T=4096, QC=1024)  # keys per batch, query rows per core
MINI_CFG = dict(T=512, QC=256)


def attention_body(ctx, tc, io, cfg):
    nc = tc.nc
    T, QC = cfg["T"], cfg["QC"]
    KT = T // 128              # key tiles
    NQH = 2 if QC >= 1024 else 1
    QW = QC // NQH             # query-half width (512 full config)
    NCH = max(1, QC // 512)    # 512-wide matmul chunks per QC
    KG = max(1, T // QC)       # K-proj groups of QC keys per e-tile
    scale = 1.0 / math.sqrt(EMB)

    pw = ctx.enter_context(tc.tile_pool(name="w", bufs=1))
    pk = ctx.enter_context(tc.tile_pool(name="kt", bufs=1))
    pv = ctx.enter_context(tc.tile_pool(name="v", bufs=1))
    pq = ctx.enter_context(tc.tile_pool(name="qt", bufs=1))
    px = ctx.enter_context(tc.tile_pool(name="x", bufs=1))

    # persistent weights / biases / constants; DMA order = need order
    bqr = pw.tile([128, CT], F32, tag="bqr", name="bqr")
    bkr = pw.tile([128, CT], F32, tag="bkr", name="bkr")
    nc.sync.dma_start(bqr[:], io["bqr"][:, :])
    nc.sync.dma_start(bkr[:], io["bkr"][:, :])
    wq = [pw.tile([128, EMB], F16, tag=f"wq{i}", name=f"wq{i}") for i in range(CT)]
    wk = [pw.tile([128, EMB], F16, tag=f"wk{i}", name=f"wk{i}") for i in range(CT)]
    wv = [pw.tile([128, EMB], F16, tag=f"wv{i}", name=f"wv{i}") for i in range(CT)]
    wu = [pw.tile([128, EMB], F16, tag=f"wu{i}", name=f"wu{i}") for i in range(CT)]
    for i in range(CT):
        nc.sync.dma_start(wq[i][:], io["wqT"][bass.ts(i, 128), :])
        nc.sync.dma_start(wk[i][:], io["wkT"][bass.ts(i, 128), :])
        nc.sync.dma_start(wv[i][:], io["wvT"][bass.ts(i, 128), :])
    x1t = [px.tile([128, QC], F16, tag=f"x1t{i}", name=f"x1t{i}") for i in range(CT)]
    for i in range(CT):
        nc.sync.dma_start(x1t[i][:], io["x1T"][bass.ts(i, 128), :])
    # x2^T lands column-block-first so K/V proj can chase the transfer
    x2t = [px.tile([128, T], F16, tag=f"x2t{i}", name=f"x2t{i}") for i in range(CT)]
    for j in range(T // 512):
        for i in range(CT):
            nc.sync.dma_start(x2t[i][:, bass.ts(j, 512)],
                              io["x2T"][bass.ts(i, 128), bass.ts(j, 512)])
    bvb = pw.tile([128, EMB], F32, tag="bvb", name="bvb")
    bub = pw.tile([128, EMB], F32, tag="bub", name="bub")
    nc.sync.dma_start(bvb[:], io["bvb"][:, :])
    nc.sync.dma_start(bub[:], io["bub"][:, :])
    for i in range(CT):
        nc.sync.dma_start(wu[i][:], io["wuT"][bass.ts(i, 128), :])
    ones = pw.tile([1, D], F16, tag="ones", name="ones")
    nc.vector.memset(ones[:], 1.0)

    # persistent K^T [emb, T], V [key, head, 65(+pad)], Q^T [emb, QC]
    kt = [pk.tile([128, T], F16, tag=f"kt{i}", name=f"kt{i}") for i in range(CT)]
    v = pv.tile([128, KT, H, 66], F16, tag="v", name="v")
    nc.vector.memset(v[:, :, :, 64:65], 1.0)
    qt = [pq.tile([128, QC], F16, tag=f"qt{i}", name=f"qt{i}") for i in range(CT)]

    # ---- pre-attention projections: Q, K e-tile 0, V -----------------
    # ACT does Q/K bias adds (idle until attention), GpSimd does V's.
    with tc.tile_pool(name="pp0", bufs=1, space="PSUM") as pp0:
        for e in range(CT):
            ps = pp0.tile([128, QC], F32, tag="qp", name=f"qps{e}", bufs=2)
            for t in range(NCH):
                for c in range(CT):
                    nc.tensor.matmul(ps[:, bass.ts(t, QC // NCH)],
                                     wq[c][:, bass.ts(e, 128)],
                                     x1t[c][:, bass.ts(t, QC // NCH)],
                                     start=(c == 0), stop=(c == CT - 1))
            nc.scalar.add(qt[e][:], ps[:], bqr[:, e:e + 1])
        for g in range(KG):
            ps = pp0.tile([128, QC], F32, tag="qp", name=f"kps0{g}", bufs=2)
            for t in range(NCH):
                for c in range(CT):
                    nc.tensor.matmul(
                        ps[:, bass.ts(t, QC // NCH)],
                        wk[c][:, 0:128],
                        x2t[c][:, bass.ds(g * QC + t * (QC // NCH), QC // NCH)],
                        start=(c == 0), stop=(c == CT - 1))
            nc.scalar.add(kt[0][:, bass.ts(g, QC)], ps[:], bkr[:, 0:1])
        for t in range(KT):
            ps = pp0.tile([128, EMB], F32, tag="vp", name=f"vps{t}", bufs=4)
            for c in range(CT):
                nc.tensor.matmul(ps[:], x2t[c][:, bass.ts(t, 128)], wv[c][:],
                                 start=(c == 0), stop=(c == CT - 1))
            nc.vector.tensor_add(
                v[:, t, :, 0:64],
                ps[:].rearrange("p (h d) -> p h d", h=H),
                bvb[:].rearrange("p (h d) -> p h d", h=H))

    # ---- attention -----------------------------------------------------
    with tc.tile_pool(name="ps_s", bufs=2, space="PSUM") as ps_s, \
         tc.tile_pool(name="ps_av", bufs=2, space="PSUM") as ps_av, \
         tc.tile_pool(name="ps_kp", bufs=1, space="PSUM") as ps_kp, \
         tc.tile_pool(name="pe", bufs=3) as pe, \
         tc.tile_pool(name="ppp", bufs=3) as ppp, \
         tc.tile_pool(name="pm", bufs=8) as pm, \
         tc.tile_pool(name="pys", bufs=6) as pys, \
         tc.tile_pool(name="prr", bufs=2) as prr, \
         tc.tile_pool(name="po", bufs=2) as po:
        yts = [py_tile for py_tile in
               (pq.tile([128, QC], F16, tag=f"yt{e}", name=f"yt{e}")
                for e in range(CT))]

        def k_proj(e):
            # trickles through 1-buffer PSUM tag during attention (DVE adds)
            for g in range(KG):
                ps = ps_kp.tile([128, QC], F32, tag="kp", name=f"kps{e}{g}")
                for t in range(NCH):
                    for c in range(CT):
                        nc.tensor.matmul(
                            ps[:, bass.ts(t, QC // NCH)],
                            wk[c][:, bass.ts(e, 128)],
                            x2t[c][:, bass.ds(g * QC + t * (QC // NCH), QC // NCH)],
                            start=(c == 0), stop=(c == CT - 1))
                nc.vector.tensor_scalar_add(kt[e][:, bass.ts(g, QC)], ps[:],
                                            bkr[:, e:e + 1])

        kq = [1, 2, 3]  # pending K-proj e-tiles, emitted between q-halves
        ysbs, rrs = {}, {}
        for pr in range(CT):        # head pair
            for qh in range(NQH):   # query half
                avs = [ps_av.tile([65, QW], F32, tag="av", name=f"av{hh}")
                       for hh in range(2)]
                for kk in range(KT):
                    mkt = pm.tile([128, QW], F16, tag="mk", name="mk")
                    nc.sync.dma_start(
                        mkt[:], io["maskT"][bass.ts(kk, 128),
                                            bass.ds(qh * QW, QW)])
                    s = ps_s.tile([128, 2 * QW], F32, tag="s", name="s")
                    for hh in range(2):
                        nc.tensor.matmul(
                            s[:, bass.ds(hh * QW, QW)],
                            kt[pr][bass.ds(64 * hh, 64), bass.ts(kk, 128)],
                            qt[pr][bass.ds(64 * hh, 64), bass.ds(qh * QW, QW)],
                            start=True, stop=True,
                            tile_position=(64 * hh, 0))
                    e16 = pe.tile([128, 2 * QW], F16, tag="E", name="e16")
                    nc.scalar.activation(e16[:], s[:], EXP, scale=scale)
                    pt = ppp.tile([128, 2 * QW], F16, tag="P", name="pt")
                    nc.vector.tensor_mul(
                        pt[:].rearrange("p (h q) -> p h q", h=2),
                        e16[:].rearrange("p (h q) -> p h q", h=2),
                        mkt[:].unsqueeze(1).broadcast_to([128, 2, QW]))
                    for hh in range(2):
                        nc.tensor.matmul(
                            avs[hh][:], v[:, kk, 2 * pr + hh, 0:65],
                            pt[:, bass.ds(hh * QW, QW)],
                            start=(kk == 0), stop=(kk == KT - 1))
                # drain r + Y^T out of PSUM so av buffers recycle fast
                for hh in range(2):
                    ysb = pys.tile([65, QW], F32, tag="ys", name="ysb")
                    nc.vector.tensor_copy(ysb[:], avs[hh][:])
                    r0 = prr.tile([1, QW], F32, tag="r0", name="r0")
                    nc.vector.tensor_copy(r0[:], ysb[64:65, :])
                    rr32 = prr.tile([1, QW], F32, tag="rr32", name="rr32")
                    nc.vector.reciprocal_approx_fast(rr32[:], r0[:])
                    rr = prr.tile([1, QW], F16, tag="rr", name="rr", bufs=4)
                    with nc.allow_low_precision(reason="fp16 recip copy ok"):
                        nc.vector.tensor_copy(rr[:], rr32[:])
                    ysbs[(qh, hh)] = ysb
                    rrs[(qh, hh)] = rr
                if kq:
                    k_proj(kq.pop(0))
            # normalize: Y^T_h / r_h via K=1 ones-matmul broadcast
            for hh in range(2):
                bc = ps_kp.tile([64, QC], F32, tag="kp", name="bc")
                for qh in range(NQH):
                    nc.tensor.matmul(bc[:, bass.ds(qh * QW, QW)], ones[:],
                                     rrs[(qh, hh)][:], start=True, stop=True)
                for qh in range(NQH):
                    nc.vector.tensor_mul(
                        yts[pr][bass.ds(64 * hh, 64), bass.ds(qh * QW, QW)],
                        ysbs[(qh, hh)][0:64, :], bc[:, bass.ds(qh * QW, QW)])

        if "dbg" in io:
            for e in range(CT):
                nc.sync.dma_start(io["dbg_qt"][bass.ts(e, 128), :], qt[e][:])
                nc.sync.dma_start(io["dbg_kt"][bass.ts(e, 128), :], kt[e][:])
                nc.sync.dma_start(io["dbg_yt"][bass.ts(e, 128), :], yts[e][:])
            for t in range(KT):
                nc.sync.dma_start(
                    io["dbg_v"][:, :].rearrange("p (a b) -> p a b", a=KT)[:, t, :],
                    v[:, t, :, :].rearrange("p a b -> p (a b)"))

        # out[q, :] = sum_e Y^T[e, q] * WuT[e, :] + bu
        pso_tag = "av" if QW * 4 == EMB * 4 else "pso"
        for qi in range(QC // 128):
            pso = ps_av.tile([128, EMB], F32, tag=pso_tag, name="pso")
            for e in range(CT):
                nc.tensor.matmul(pso[:], yts[e][:, bass.ts(qi, 128)], wu[e][:],
                                 start=(e == 0), stop=(e == CT - 1))
            osb = po.tile([128, EMB], F32, tag="o", name="osb")
            nc.vector.tensor_add(osb[:], pso[:], bub[:])
            nc.sync.dma_start(io["out"][bass.ts(qi, 128), :], osb[:])


def build(cfg, num_devices=8, dbg=False):
    T, QC = cfg["T"], cfg["QC"]
    nc = bacc.Bacc("TRN2", target_bir_lowering=False, debug=False,
                   num_devices=num_devices)
    io = {
        "x1T": nc.dram_tensor("x1T", [EMB, QC], F16, kind="ExternalInput").ap(),
        "x2T": nc.dram_tensor("x2T", [EMB, T], F16, kind="ExternalInput").ap(),
        "maskT": nc.dram_tensor("maskT", [T, QC], F16, kind="ExternalInput").ap(),
        "wqT": nc.dram_tensor("wqT", [EMB, EMB], F16, kind="ExternalInput").ap(),
        "wkT": nc.dram_tensor("wkT", [EMB, EMB], F16, kind="ExternalInput").ap(),
        "wvT": nc.dram_tensor("wvT", [EMB, EMB], F16, kind="ExternalInput").ap(),
        "wuT": nc.dram_tensor("wuT", [EMB, EMB], F16, kind="ExternalInput").ap(),
        "bqr": nc.dram_tensor("bqr", [128, CT], F32, kind="ExternalInput").ap(),
        "bkr": nc.dram_tensor("bkr", [128, CT], F32, kind="ExternalInput").ap(),
        "bvb": nc.dram_tensor("bvb", [128, EMB], F32, kind="ExternalInput").ap(),
        "bub": nc.dram_tensor("bub", [128, EMB], F32, kind="ExternalInput").ap(),
        "out": nc.dram_tensor("out", [QC, EMB], F32, kind="ExternalOutput").ap(),
    }
    if dbg:
        KT = T // 128
        io["dbg"] = True
        io["dbg_qt"] = nc.dram_tensor("dbg_qt", [EMB, QC], F16, kind="ExternalOutput").ap()
        io["dbg_kt"] = nc.dram_tensor("dbg_kt", [EMB, T], F16, kind="ExternalOutput").ap()
        io["dbg_yt"] = nc.dram_tensor("dbg_yt", [EMB, QC], F16, kind="ExternalOutput").ap()
        io["dbg_v"] = nc.dram_tensor("dbg_v", [128, KT * H * 66], F16, kind="ExternalOutput").ap()
    with tile.TileContext(nc) as tc:
        with ExitStack() as ctx:
            attention_body(ctx, tc, io, cfg)
    nc.compile()
    return nc


def host_prep(x1, x2, mask, Wq, bq, Wk, bk, Wv, bv, Wu, bu, cfg):
    """Build the 8 per-core input maps from full inputs."""
    T, QC = cfg["T"], cfg["QC"]
    shared = {
        "wqT": np.ascontiguousarray(Wq.T).astype(np.float16),
        "wkT": np.ascontiguousarray(Wk.T).astype(np.float16),
        "wvT": np.ascontiguousarray(Wv.T).astype(np.float16),
        "wuT": np.ascontiguousarray(Wu.T).astype(np.float16),
        "bqr": np.ascontiguousarray(bq.reshape(CT, 128).T).astype(np.float32),
        "bkr": np.ascontiguousarray(bk.reshape(CT, 128).T).astype(np.float32),
        "bvb": np.ascontiguousarray(np.broadcast_to(bv, (128, EMB))).astype(np.float32),
        "bub": np.ascontiguousarray(np.broadcast_to(bu, (128, EMB))).astype(np.float32),
    }
    x2T = [x2[b].T.astype(np.float16) for b in range(x1.shape[0])]
    in_maps = []
    n_cores = (x1.shape[0] * x1.shape[1]) // QC
    per_b = x1.shape[1] // QC
    for c in range(n_cores):
        b, q0 = c // per_b, (c % per_b) * QC
        in_maps.append(dict(
            shared,
            x1T=x1[b, q0:q0 + QC, :].T.astype(np.float16),
            x2T=x2T[b],
            maskT=mask[b, q0:q0 + QC, :].T.astype(np.float16),
        ))
    return in_maps


_NC_CACHE = {}


def kernel(x1, x2, mask, Wq, bq, Wk, bk, Wv, bv, Wu, bu):
    cfg = FULL_CFG
    B, TQ, _ = x1.shape
    in_maps = host_prep(np.asarray(x1, np.float32), np.asarray(x2, np.float32),
                        np.asarray(mask), np.asarray(Wq, np.float32),
                        np.asarray(bq, np.float32), np.asarray(Wk, np.float32),
                        np.asarray(bk, np.float32), np.asarray(Wv, np.float32),
                        np.asarray(bv, np.float32), np.asarray(Wu, np.float32),
                        np.asarray(bu, np.float32), cfg)
    key = (cfg["T"], cfg["QC"])
    if key not in _NC_CACHE:
        _NC_CACHE[key] = build(cfg)
    nc = _NC_CACHE[key]
    res = run_bass_kernel_spmd(nc, in_maps, core_ids=list(range(8)),
                               trace=bool(os.environ.get("KERNEL_TRACE")))
    if os.environ.get("KERNEL_TRACE"):
        kernel.last_exec_ns = res.exec_time_ns
        kernel.last_results = res
    out = np.empty((B, TQ, EMB), np.float32)
    per_b = TQ // cfg["QC"]
    for c in range(8):
        b, q0 = c // per_b, (c % per_b) * cfg["QC"]
        out[b, q0:q0 + cfg["QC"], :] = res.results[c]["out"]
    return out


# revision 17
# speedup vs baseline: 1.2185x; 1.0041x over previous
"""Multi-head cross attention on 8 trn2 NeuronCores.

Problem: B=2, T=4096, EMB=512, H=8 heads (head dim 64), fp32 I/O.
  q = x1 @ Wq.T + bq ; k,v from x2 ; S = q k^T / sqrt(512) ;
  softmax over keys with -1e10 masking ; out = (A v) @ Wu.T + bu.

Sharding: core c handles batch b = c//4 and query rows
[1024*(c%4), 1024*(c%4+1)).  Each core computes K,V for its batch in
full (4-way duplication), its own Q chunk, attention, and out-proj.

Device-side design (v2) — the kernel is ACT(exp)-bound, so everything
is organized to keep the Activation engine streaming EXPs gaplessly:
  - Scores computed TRANSPOSED, S^T[key, query] (head dim on PE
    partitions), from host-pre-transposed fp16 inputs.  2 heads packed
    per scores pass via tile_position row-tiling.
  - Queries processed in two 512-wide halves so each (head-pair, kk)
    score tile is [128, 1024] = 2 PSUM banks; tag-rotated double
    buffering (4 banks) lets EXP(kk+1) run while scores(kk+2) fill.
  - AV accumulates per head in [65, QW] PSUM (ones column in V gives
    softmax denominators); 2 heads x 1 bank, rotated (2 banks).
  - K projection for e-tiles 1-3 trickles through a single [128,1024]
    PSUM tag (2 banks) DURING attention (DVE bias adds); e-tile 0, Q
    and V are computed up front with ACT/GpSimd bias adds (ACT is idle
    pre-attention; Identity and Exp share an activation table so no
    table reloads).  x2^T is DMA'd column-block-first so K/V proj
    start before the transfer completes.
  - Normalization: r rows copied out via GpSimd, reciprocal on DVE,
    K=1 ones-matmul broadcast, final scale on DVE; the broadcast tile
    reuses the K-proj PSUM tag, out-proj PSUM reuses the av tag, so
    total PSUM = 4+2+2 = 8 banks with no pool barriers inside the
    attention phase.
"""
import math
import os
from contextlib import ExitStack

import numpy as np

import concourse.bass as bass
import concourse.bacc as bacc
import concourse.tile as tile
import concourse.mybir as mybir
from concourse.bass_utils import run_bass_kernel_spmd

F16 = mybir.dt.float16
F32 = mybir.dt.float32
EXP = mybir.ActivationFunctionType.Exp

EMB, H, D, CT = 512, 8, 64, 4  # emb, heads, head dim, emb/128

FULL_CFG = dict(T=4096, QC=1024)  # keys per batch, query rows per core
MINI_CFG = dict(T=512, QC=256)


def attention_body(ctx, tc, io, cfg):
    nc = tc.nc
    T, QC = cfg["T"], cfg["QC"]
    KT = T // 128              # key tiles
    NQH = 2 if QC >= 1024 else 1
    QW = QC // NQH             # query-half width (512 full config)
    NCH = max(1, QC // 512)    # 512-wide matmul chunks per QC
    KG = max(1, T // QC)       # K-proj groups of QC keys per e-tile
    scale = 1.0 / math.sqrt(EMB)

    pw = ctx.enter_context(tc.tile_pool(name="w", bufs=1))
    pk = ctx.enter_context(tc.tile_pool(name="kt", bufs=1))
    pv = ctx.enter_context(tc.tile_pool(name="v", bufs=1))
    pq = ctx.enter_context(tc.tile_pool(name="qt", bufs=1))
    px = ctx.enter_context(tc.tile_pool(name="x", bufs=1))

    # persistent weights / biases / constants; DMA order = need order
    bqr = pw.tile([128, CT], F32, tag="bqr", name="bqr")
    bkr = pw.tile([128, CT], F32, tag="bkr", name="bkr")
    nc.sync.dma_start(bqr[:], io["bqr"][:, :])
    nc.sync.dma_start(bkr[:], io["bkr"][:, :])
    wq = [pw.tile([128, EMB], F16, tag=f"wq{i}", name=f"wq{i}") for i in range(CT)]
    wk = [pw.tile([128, EMB], F16, tag=f"wk{i}", name=f"wk{i}") for i in range(CT)]
    wv = [pw.tile([128, EMB], F16, tag=f"wv{i}", name=f"wv{i}") for i in range(CT)]
    wu = [pw.tile([128, EMB], F16, tag=f"wu{i}", name=f"wu{i}") for i in range(CT)]
    for i in range(CT):
        nc.sync.dma_start(wq[i][:], io["wqT"][bass.ts(i, 128), :])
        nc.sync.dma_start(wk[i][:], io["wkT"][bass.ts(i, 128), :])
        nc.sync.dma_start(wv[i][:], io["wvT"][bass.ts(i, 128), :])
    x1t = [px.tile([128, QC], F16, tag=f"x1t{i}", name=f"x1t{i}") for i in range(CT)]
    for i in range(CT):
        nc.sync.dma_start(x1t[i][:], io["x1T"][bass.ts(i, 128), :])
    # x2^T lands column-block-first so K/V proj can chase the transfer
    x2t = [px.tile([128, T], F16, tag=f"x2t{i}", name=f"x2t{i}") for i in range(CT)]
    for j in range(T // 512):
        for i in range(CT):
            nc.sync.dma_start(x2t[i][:, bass.ts(j, 512)],
                              io["x2T"][bass.ts(i, 128), bass.ts(j, 512)])
    bub = pw.tile([128, EMB], F32, tag="bub", name="bub")
    nc.sync.dma_start(bub[:], io["bub"][:, :])
    for i in range(CT):
        nc.sync.dma_start(wu[i][:], io["wuT"][bass.ts(i, 128), :])
    ones = pw.tile([1, D], F16, tag="ones", name="ones")
    nc.vector.memset(ones[:], 1.0)

    # persistent K^T [emb, T], V [key, head, 65(+pad)], Q^T [emb, QC]
    kt = [pk.tile([128, T], F16, tag=f"kt{i}", name=f"kt{i}") for i in range(CT)]
    v = pv.tile([128, KT, H, 66], F16, tag="v", name="v")
    nc.vector.memset(v[:, :, :, 64:65], 1.0)
    qt = [pq.tile([128, QC], F16, tag=f"qt{i}", name=f"qt{i}") for i in range(CT)]

    # ---- pre-attention projections: V, Q, K e-tile 0 -----------------
    # bv is folded into bub on the host (out = y0@Wu.T + (bu + Wu@bv)),
    # so V just drains PSUM -> fp16 interleaved, split across ACT + DVE.
    with tc.tile_pool(name="pp0", bufs=1, space="PSUM") as pp0:
        for t in range(KT):
            ps = pp0.tile([128, EMB], F32, tag="vp", name=f"vps{t}", bufs=4)
            for c in range(CT):
                nc.tensor.matmul(ps[:], x2t[c][:, bass.ts(t, 128)], wv[c][:],
                                 start=(c == 0), stop=(c == CT - 1))
            vdst = v[:, t, :, 0:64]
            psr = ps[:].rearrange("p (h d) -> p h d", h=H)
            if t % 2 == 0:
                nc.scalar.copy(vdst, psr)
            else:
                nc.vector.tensor_copy(vdst, psr)
        for e in range(CT):
            ps = pp0.tile([128, QC], F32, tag="qp", name=f"qps{e}", bufs=2)
            for t in range(NCH):
                for c in range(CT):
                    nc.tensor.matmul(ps[:, bass.ts(t, QC // NCH)],
                                     wq[c][:, bass.ts(e, 128)],
                                     x1t[c][:, bass.ts(t, QC // NCH)],
                                     start=(c == 0), stop=(c == CT - 1))
            nc.scalar.add(qt[e][:], ps[:], bqr[:, e:e + 1])
        for g in range(KG):
            ps = pp0.tile([128, QC], F32, tag="qp", name=f"kps0{g}", bufs=2)
            for t in range(NCH):
                for c in range(CT):
                    nc.tensor.matmul(
                        ps[:, bass.ts(t, QC // NCH)],
                        wk[c][:, 0:128],
                        x2t[c][:, bass.ds(g * QC + t * (QC // NCH), QC // NCH)],
                        start=(c == 0), stop=(c == CT - 1))
            nc.scalar.add(kt[0][:, bass.ts(g, QC)], ps[:], bkr[:, 0:1])

    # ---- attention -----------------------------------------------------
    with tc.tile_pool(name="ps_s", bufs=2, space="PSUM") as ps_s, \
         tc.tile_pool(name="ps_av", bufs=2, space="PSUM") as ps_av, \
         tc.tile_pool(name="ps_kp", bufs=1, space="PSUM") as ps_kp, \
         tc.tile_pool(name="pe", bufs=3) as pe, \
         tc.tile_pool(name="ppp", bufs=3) as ppp, \
         tc.tile_pool(name="pm", bufs=14) as pm, \
         tc.tile_pool(name="pys", bufs=6) as pys, \
         tc.tile_pool(name="prr", bufs=2) as prr, \
         tc.tile_pool(name="po", bufs=2) as po:
        yts = [py_tile for py_tile in
               (pq.tile([128, QC], F16, tag=f"yt{e}", name=f"yt{e}")
                for e in range(CT))]

        def k_proj(e):
            # trickles through 1-buffer PSUM tag during attention (DVE adds)
            for g in range(KG):
                ps = ps_kp.tile([128, QC], F32, tag="kp", name=f"kps{e}{g}")
                for t in range(NCH):
                    for c in range(CT):
                        nc.tensor.matmul(
                            ps[:, bass.ts(t, QC // NCH)],
                            wk[c][:, bass.ts(e, 128)],
                            x2t[c][:, bass.ds(g * QC + t * (QC // NCH), QC // NCH)],
                            start=(c == 0), stop=(c == CT - 1))
                nc.vector.tensor_scalar_add(kt[e][:, bass.ts(g, QC)], ps[:],
                                            bkr[:, e:e + 1])

        kq = [1, 2, 3]  # pending K-proj e-tiles, emitted between q-halves
        ysbs, rrs = {}, {}
        for pr in range(CT):        # head pair
            for qh in range(NQH):   # query half
                avs = [ps_av.tile([65, QW], F32, tag="av", name=f"av{hh}")
                       for hh in range(2)]
                for kk in range(KT):
                    mkt = pm.tile([128, QW], F16, tag="mk", name="mk")
                    # issued from the otherwise-idle GpSimd stream so mask
                    # prefetch is not serialized behind the sync stream's
                    # pool-release waits
                    nc.gpsimd.dma_start(
                        mkt[:], io["maskT"][bass.ts(kk, 128),
                                            bass.ds(qh * QW, QW)])
                    s = ps_s.tile([128, 2 * QW], F32, tag="s", name="s")
                    for hh in range(2):
                        nc.tensor.matmul(
                            s[:, bass.ds(hh * QW, QW)],
                            kt[pr][bass.ds(64 * hh, 64), bass.ts(kk, 128)],
                            qt[pr][bass.ds(64 * hh, 64), bass.ds(qh * QW, QW)],
                            start=True, stop=True,
                            tile_position=(64 * hh, 0))
                    e16 = pe.tile([128, 2 * QW], F16, tag="E", name="e16")
                    nc.scalar.activation(e16[:], s[:], EXP, scale=scale)
                    pt = ppp.tile([128, 2 * QW], F16, tag="P", name="pt")
                    nc.vector.tensor_mul(
                        pt[:].rearrange("p (h q) -> p h q", h=2),
                        e16[:].rearrange("p (h q) -> p h q", h=2),
                        mkt[:].unsqueeze(1).broadcast_to([128, 2, QW]))
                    for hh in range(2):
                        nc.tensor.matmul(
                            avs[hh][:], v[:, kk, 2 * pr + hh, 0:65],
                            pt[:, bass.ds(hh * QW, QW)],
                            start=(kk == 0), stop=(kk == KT - 1))
                # drain r + Y^T out of PSUM so av buffers recycle fast
                for hh in range(2):
                    ysb = pys.tile([65, QW], F32, tag="ys", name="ysb")
                    nc.vector.tensor_copy(ysb[:], avs[hh][:])
                    r0 = prr.tile([1, QW], F32, tag="r0", name="r0")
                    nc.vector.tensor_copy(r0[:], ysb[64:65, :])
                    rr32 = prr.tile([1, QW], F32, tag="rr32", name="rr32")
                    nc.vector.reciprocal_approx_fast(rr32[:], r0[:])
                    rr = prr.tile([1, QW], F16, tag="rr", name="rr", bufs=4)
                    with nc.allow_low_precision(reason="fp16 recip copy ok"):
                        nc.vector.tensor_copy(rr[:], rr32[:])
                    ysbs[(qh, hh)] = ysb
                    rrs[(qh, hh)] = rr
                if kq:
                    k_proj(kq.pop(0))
            # normalize: Y^T_h / r_h via K=1 ones-matmul broadcast
            for hh in range(2):
                bc = ps_kp.tile([64, QC], F32, tag="kp", name="bc")
                for qh in range(NQH):
                    nc.tensor.matmul(bc[:, bass.ds(qh * QW, QW)], ones[:],
                                     rrs[(qh, hh)][:], start=True, stop=True)
                for qh in range(NQH):
                    nc.vector.tensor_mul(
                        yts[pr][bass.ds(64 * hh, 64), bass.ds(qh * QW, QW)],
                        ysbs[(qh, hh)][0:64, :], bc[:, bass.ds(qh * QW, QW)])

        if "dbg" in io:
            for e in range(CT):
                nc.sync.dma_start(io["dbg_qt"][bass.ts(e, 128), :], qt[e][:])
                nc.sync.dma_start(io["dbg_kt"][bass.ts(e, 128), :], kt[e][:])
                nc.sync.dma_start(io["dbg_yt"][bass.ts(e, 128), :], yts[e][:])
            for t in range(KT):
                nc.sync.dma_start(
                    io["dbg_v"][:, :].rearrange("p (a b) -> p a b", a=KT)[:, t, :],
                    v[:, t, :, :].rearrange("p a b -> p (a b)"))

        # out[q, :] = sum_e Y^T[e, q] * WuT[e, :] + bu
        pso_tag = "av" if QW * 4 == EMB * 4 else "pso"
        for qi in range(QC // 128):
            pso = ps_av.tile([128, EMB], F32, tag=pso_tag, name="pso")
            for e in range(CT):
                nc.tensor.matmul(pso[:], yts[e][:, bass.ts(qi, 128)], wu[e][:],
                                 start=(e == 0), stop=(e == CT - 1))
            osb = po.tile([128, EMB], F32, tag="o", name="osb")
            nc.vector.tensor_add(osb[:], pso[:], bub[:])
            for hf in range(2):
                nc.sync.dma_start(
                    io["out"][bass.ds(qi * 128 + hf * 64, 64), :],
                    osb[bass.ds(hf * 64, 64), :])


def build(cfg, num_devices=8, dbg=False):
    T, QC = cfg["T"], cfg["QC"]
    nc = bacc.Bacc("TRN2", target_bir_lowering=False, debug=False,
                   num_devices=num_devices)
    io = {
        "x1T": nc.dram_tensor("x1T", [EMB, QC], F16, kind="ExternalInput").ap(),
        "x2T": nc.dram_tensor("x2T", [EMB, T], F16, kind="ExternalInput").ap(),
        "maskT": nc.dram_tensor("maskT", [T, QC], F16, kind="ExternalInput").ap(),
        "wqT": nc.dram_tensor("wqT", [EMB, EMB], F16, kind="ExternalInput").ap(),
        "wkT": nc.dram_tensor("wkT", [EMB, EMB], F16, kind="ExternalInput").ap(),
        "wvT": nc.dram_tensor("wvT", [EMB, EMB], F16, kind="ExternalInput").ap(),
        "wuT": nc.dram_tensor("wuT", [EMB, EMB], F16, kind="ExternalInput").ap(),
        "bqr": nc.dram_tensor("bqr", [128, CT], F32, kind="ExternalInput").ap(),
        "bkr": nc.dram_tensor("bkr", [128, CT], F32, kind="ExternalInput").ap(),
        "bub": nc.dram_tensor("bub", [128, EMB], F32, kind="ExternalInput").ap(),
        "out": nc.dram_tensor("out", [QC, EMB], F32, kind="ExternalOutput").ap(),
    }
    if dbg:
        KT = T // 128
        io["dbg"] = True
        io["dbg_qt"] = nc.dram_tensor("dbg_qt", [EMB, QC], F16, kind="ExternalOutput").ap()
        io["dbg_kt"] = nc.dram_tensor("dbg_kt", [EMB, T], F16, kind="ExternalOutput").ap()
        io["dbg_yt"] = nc.dram_tensor("dbg_yt", [EMB, QC], F16, kind="ExternalOutput").ap()
        io["dbg_v"] = nc.dram_tensor("dbg_v", [128, KT * H * 66], F16, kind="ExternalOutput").ap()
    with tile.TileContext(nc) as tc:
        with ExitStack() as ctx:
            attention_body(ctx, tc, io, cfg)
    nc.compile()
    return nc


def host_prep(x1, x2, mask, Wq, bq, Wk, bk, Wv, bv, Wu, bu, cfg):
    """Build the 8 per-core input maps from full inputs."""
    T, QC = cfg["T"], cfg["QC"]
    shared = {
        "wqT": np.ascontiguousarray(Wq.T).astype(np.float16),
        "wkT": np.ascontiguousarray(Wk.T).astype(np.float16),
        "wvT": np.ascontiguousarray(Wv.T).astype(np.float16),
        "wuT": np.ascontiguousarray(Wu.T).astype(np.float16),
        "bqr": np.ascontiguousarray(bq.reshape(CT, 128).T).astype(np.float32),
        "bkr": np.ascontiguousarray(bk.reshape(CT, 128).T).astype(np.float32),
        # bv folded into the output bias: out = y0@Wu.T + (bu + Wu@bv)
        "bub": np.ascontiguousarray(
            np.broadcast_to(bu + Wu @ bv, (128, EMB))).astype(np.float32),
    }
    x2T = [x2[b].T.astype(np.float16) for b in range(x1.shape[0])]
    in_maps = []
    n_cores = (x1.shape[0] * x1.shape[1]) // QC
    per_b = x1.shape[1] // QC
    for c in range(n_cores):
        b, q0 = c // per_b, (c % per_b) * QC
        in_maps.append(dict(
            shared,
            x1T=x1[b, q0:q0 + QC, :].T.astype(np.float16),
            x2T=x2T[b],
            maskT=mask[b, q0:q0 + QC, :].T.astype(np.float16),
        ))
    return in_maps


_NC_CACHE = {}


def kernel(x1, x2, mask, Wq, bq, Wk, bk, Wv, bv, Wu, bu):
    cfg = FULL_CFG
    B, TQ, _ = x1.shape
    in_maps = host_prep(np.asarray(x1, np.float32), np.asarray(x2, np.float32),
                        np.asarray(mask), np.asarray(Wq, np.float32),
                        np.asarray(bq, np.float32), np.asarray(Wk, np.float32),
                        np.asarray(bk, np.float32), np.asarray(Wv, np.float32),
                        np.asarray(bv, np.float32), np.asarray(Wu, np.float32),
                        np.asarray(bu, np.float32), cfg)
    key = (cfg["T"], cfg["QC"])
    if key not in _NC_CACHE:
        _NC_CACHE[key] = build(cfg)
    nc = _NC_CACHE[key]
    res = run_bass_kernel_spmd(nc, in_maps, core_ids=list(range(8)),
                               trace=bool(os.environ.get("KERNEL_TRACE")))
    if os.environ.get("KERNEL_TRACE"):
        kernel.last_exec_ns = res.exec_time_ns
        kernel.last_results = res
    out = np.empty((B, TQ, EMB), np.float32)
    per_b = TQ // cfg["QC"]
    for c in range(8):
        b, q0 = c // per_b, (c % per_b) * cfg["QC"]
        out[b, q0:q0 + cfg["QC"], :] = res.results[c]["out"]
    return out


# revision 22
# speedup vs baseline: 1.2385x; 1.0163x over previous
"""Multi-head cross attention on 8 trn2 NeuronCores.

Problem: B=2, T=4096, EMB=512, H=8 heads (head dim 64), fp32 I/O.
  q = x1 @ Wq.T + bq ; k,v from x2 ; S = q k^T / sqrt(512) ;
  softmax over keys with -1e10 masking ; out = (A v) @ Wu.T + bu.

Sharding: core c handles batch b = c//4 and query rows
[1024*(c%4), 1024*(c%4+1)).  Each core computes K,V for its batch in
full (4-way duplication), its own Q chunk, attention, and out-proj.

Device-side design (v2) — the kernel is ACT(exp)-bound, so everything
is organized to keep the Activation engine streaming EXPs gaplessly:
  - Scores computed TRANSPOSED, S^T[key, query] (head dim on PE
    partitions), from host-pre-transposed fp16 inputs.  2 heads packed
    per scores pass via tile_position row-tiling.
  - Queries processed in two 512-wide halves so each (head-pair, kk)
    score tile is [128, 1024] = 2 PSUM banks; tag-rotated double
    buffering (4 banks) lets EXP(kk+1) run while scores(kk+2) fill.
  - AV accumulates per head in [65, QW] PSUM (ones column in V gives
    softmax denominators); 2 heads x 1 bank, rotated (2 banks).
  - K projection for e-tiles 1-3 trickles through a single [128,1024]
    PSUM tag (2 banks) DURING attention (DVE bias adds); e-tile 0, Q
    and V are computed up front with ACT/GpSimd bias adds (ACT is idle
    pre-attention; Identity and Exp share an activation table so no
    table reloads).  x2^T is DMA'd column-block-first so K/V proj
    start before the transfer completes.
  - Normalization: r rows copied out via GpSimd, reciprocal on DVE,
    K=1 ones-matmul broadcast, final scale on DVE; the broadcast tile
    reuses the K-proj PSUM tag, out-proj PSUM reuses the av tag, so
    total PSUM = 4+2+2 = 8 banks with no pool barriers inside the
    attention phase.
"""
import math
import os
from contextlib import ExitStack

import numpy as np

import concourse.bass as bass
import concourse.bacc as bacc
import concourse.tile as tile
import concourse.mybir as mybir
from concourse.bass_utils import run_bass_kernel_spmd

F16 = mybir.dt.float16
F32 = mybir.dt.float32
EXP = mybir.ActivationFunctionType.Exp

EMB, H, D, CT = 512, 8, 64, 4  # emb, heads, head dim, emb/128

FULL_CFG = dict(T=4096, QC=1024)  # keys per batch, query rows per core
MINI_CFG = dict(T=512, QC=256)


def attention_body(ctx, tc, io, cfg):
    nc = tc.nc
    T, QC = cfg["T"], cfg["QC"]
    KT = T // 128              # key tiles
    NQH = 2 if QC >= 1024 else 1
    QW = QC // NQH             # query-half width (512 full config)
    NCH = max(1, QC // 512)    # 512-wide matmul chunks per QC
    KG = max(1, T // QC)       # K-proj groups of QC keys per e-tile
    scale = 1.0 / math.sqrt(EMB)

    pw = ctx.enter_context(tc.tile_pool(name="w", bufs=1))
    pk = ctx.enter_context(tc.tile_pool(name="kt", bufs=1))
    pv = ctx.enter_context(tc.tile_pool(name="v", bufs=1))
    pq = ctx.enter_context(tc.tile_pool(name="qt", bufs=1))
    px = ctx.enter_context(tc.tile_pool(name="x", bufs=1))

    # persistent weights / biases / constants; DMA order = need order
    bqr = pw.tile([128, CT], F32, tag="bqr", name="bqr")
    bkr = pw.tile([128, CT], F32, tag="bkr", name="bkr")
    nc.sync.dma_start(bqr[:], io["bqr"][:, :])
    nc.sync.dma_start(bkr[:], io["bkr"][:, :])
    wq = [pw.tile([128, EMB], F16, tag=f"wq{i}", name=f"wq{i}") for i in range(CT)]
    wk = [pw.tile([128, EMB], F16, tag=f"wk{i}", name=f"wk{i}") for i in range(CT)]
    wv = [pw.tile([128, EMB], F16, tag=f"wv{i}", name=f"wv{i}") for i in range(CT)]
    wu = [pw.tile([128, EMB], F16, tag=f"wu{i}", name=f"wu{i}") for i in range(CT)]
    for i in range(CT):
        nc.sync.dma_start(wq[i][:], io["wqT"][bass.ts(i, 128), :])
        nc.sync.dma_start(wk[i][:], io["wkT"][bass.ts(i, 128), :])
        nc.sync.dma_start(wv[i][:], io["wvT"][bass.ts(i, 128), :])
    x1t = [px.tile([128, QC], F16, tag=f"x1t{i}", name=f"x1t{i}") for i in range(CT)]
    for i in range(CT):
        nc.sync.dma_start(x1t[i][:], io["x1T"][bass.ts(i, 128), :])
    # x2^T lands column-block-first so K/V proj can chase the transfer
    x2t = [px.tile([128, T], F16, tag=f"x2t{i}", name=f"x2t{i}") for i in range(CT)]
    XJ = max(1, T // 1024)  # 2 KiB-per-partition chunks for DMA efficiency
    for j in range(XJ):
        for i in range(CT):
            nc.sync.dma_start(x2t[i][:, bass.ts(j, T // XJ)],
                              io["x2T"][bass.ts(i, 128), bass.ts(j, T // XJ)])
    bub = pw.tile([128, EMB], F32, tag="bub", name="bub")
    nc.sync.dma_start(bub[:], io["bub"][:, :])
    for i in range(CT):
        nc.sync.dma_start(wu[i][:], io["wuT"][bass.ts(i, 128), :])
    ones = pw.tile([1, D], F16, tag="ones", name="ones")
    nc.vector.memset(ones[:], 1.0)

    # persistent K^T [emb, T], V [key, head, 65(+pad)], Q^T [emb, QC]
    kt = [pk.tile([128, T], F16, tag=f"kt{i}", name=f"kt{i}") for i in range(CT)]
    v = pv.tile([128, KT, H, 66], F16, tag="v", name="v")
    nc.vector.memset(v[:, :, :, 64:65], 1.0)
    qt = [pq.tile([128, QC], F16, tag=f"qt{i}", name=f"qt{i}") for i in range(CT)]

    # ---- pre-attention projections: V, Q, K e-tile 0 -----------------
    # bv is folded into bub on the host (out = y0@Wu.T + (bu + Wu@bv)),
    # so V just drains PSUM -> fp16 interleaved, split across ACT + DVE.
    with tc.tile_pool(name="pp0", bufs=1, space="PSUM") as pp0:
        for t in range(KT):
            ps = pp0.tile([128, EMB], F32, tag="vp", name=f"vps{t}", bufs=4)
            for c in range(CT):
                nc.tensor.matmul(ps[:], x2t[c][:, bass.ts(t, 128)], wv[c][:],
                                 start=(c == 0), stop=(c == CT - 1))
            vdst = v[:, t, :, 0:64]
            psr = ps[:].rearrange("p (h d) -> p h d", h=H)
            if t % 2 == 0:
                nc.scalar.copy(vdst, psr)
            else:
                nc.vector.tensor_copy(vdst, psr)
        for e in range(CT):
            ps = pp0.tile([128, QC], F32, tag="qp", name=f"qps{e}", bufs=2)
            for t in range(NCH):
                for c in range(CT):
                    nc.tensor.matmul(ps[:, bass.ts(t, QC // NCH)],
                                     wq[c][:, bass.ts(e, 128)],
                                     x1t[c][:, bass.ts(t, QC // NCH)],
                                     start=(c == 0), stop=(c == CT - 1))
            nc.scalar.add(qt[e][:], ps[:], bqr[:, e:e + 1])
        for g in range(KG):
            ps = pp0.tile([128, QC], F32, tag="qp", name=f"kps0{g}", bufs=2)
            for t in range(NCH):
                for c in range(CT):
                    nc.tensor.matmul(
                        ps[:, bass.ts(t, QC // NCH)],
                        wk[c][:, 0:128],
                        x2t[c][:, bass.ds(g * QC + t * (QC // NCH), QC // NCH)],
                        start=(c == 0), stop=(c == CT - 1))
            nc.scalar.add(kt[0][:, bass.ts(g, QC)], ps[:], bkr[:, 0:1])

    # ---- attention -----------------------------------------------------
    with tc.tile_pool(name="ps_s", bufs=2, space="PSUM") as ps_s, \
         tc.tile_pool(name="ps_av", bufs=2, space="PSUM") as ps_av, \
         tc.tile_pool(name="ps_kp", bufs=1, space="PSUM") as ps_kp, \
         tc.tile_pool(name="pe", bufs=5) as pe, \
         tc.tile_pool(name="ppp", bufs=3) as ppp, \
         tc.tile_pool(name="pm", bufs=14) as pm, \
         tc.tile_pool(name="pys", bufs=6) as pys, \
         tc.tile_pool(name="prr", bufs=2) as prr, \
         tc.tile_pool(name="po", bufs=2) as po:
        yts = [py_tile for py_tile in
               (pq.tile([128, QC], F16, tag=f"yt{e}", name=f"yt{e}")
                for e in range(CT))]

        def k_proj(e):
            # trickles through 1-buffer PSUM tag during attention (DVE adds)
            for g in range(KG):
                ps = ps_kp.tile([128, QC], F32, tag="kp", name=f"kps{e}{g}")
                for t in range(NCH):
                    for c in range(CT):
                        nc.tensor.matmul(
                            ps[:, bass.ts(t, QC // NCH)],
                            wk[c][:, bass.ts(e, 128)],
                            x2t[c][:, bass.ds(g * QC + t * (QC // NCH), QC // NCH)],
                            start=(c == 0), stop=(c == CT - 1))
                nc.vector.tensor_scalar_add(kt[e][:, bass.ts(g, QC)], ps[:],
                                            bkr[:, e:e + 1])

        kq = [1, 2, 3]  # pending K-proj e-tiles, emitted between units
        LAG = 2         # mul/AV trail scores/EXP by LAG steps (program order)
        units = [(pr, qh) for pr in range(CT) for qh in range(NQH)]
        NU = len(units)
        avs = {}        # unit -> [av_h0, av_h1]
        ysbs, rrs = {}, {}
        pend = []       # (unit, kk, e16, mkt) awaiting mul+AV

        def mul_and_av(u, kk, e16, mkt):
            pr, qh = units[u]
            if kk == 0:
                avs[u] = [ps_av.tile([65, QW], F32, tag="av", name=f"av{hh}")
                          for hh in range(2)]
            pt = ppp.tile([128, 2 * QW], F16, tag="P", name="pt")
            nc.vector.tensor_mul(
                pt[:].rearrange("p (h q) -> p h q", h=2),
                e16[:].rearrange("p (h q) -> p h q", h=2),
                mkt[:].unsqueeze(1).broadcast_to([128, 2, QW]))
            for hh in range(2):
                nc.tensor.matmul(
                    avs[u][hh][:], v[:, kk, 2 * pr + hh, 0:65],
                    pt[:, bass.ds(hh * QW, QW)],
                    start=(kk == 0), stop=(kk == KT - 1))
            if kk == KT - 1:
                finish_unit(u)

        def finish_unit(u):
            # drain r + Y^T out of PSUM so av buffers recycle fast; the
            # last unit's drains go to ACT (idle once EXPs are done) to
            # shorten the serial DVE tail
            pr, qh = units[u]
            for hh in range(2):
                ysb = pys.tile([65, QW], F32, tag="ys", name="ysb")
                if u == NU - 1:
                    nc.scalar.copy(ysb[:], avs[u][hh][:])
                else:
                    nc.vector.tensor_copy(ysb[:], avs[u][hh][:])
                r0 = prr.tile([1, QW], F32, tag="r0", name="r0")
                nc.vector.tensor_copy(r0[:], ysb[64:65, :])
                rr32 = prr.tile([1, QW], F32, tag="rr32", name="rr32")
                nc.vector.reciprocal_approx_fast(rr32[:], r0[:])
                rr = prr.tile([1, QW], F16, tag="rr", name="rr", bufs=4)
                with nc.allow_low_precision(reason="fp16 recip copy ok"):
                    nc.vector.tensor_copy(rr[:], rr32[:])
                ysbs[(qh, hh)] = ysb
                rrs[(qh, hh)] = rr
            if qh == NQH - 1:
                # normalize: Y^T_h / r_h via K=1 ones-matmul broadcast
                for hh in range(2):
                    bc = ps_kp.tile([64, QC], F32, tag="kp", name="bc")
                    for q2 in range(NQH):
                        nc.tensor.matmul(bc[:, bass.ds(q2 * QW, QW)], ones[:],
                                         rrs[(q2, hh)][:], start=True, stop=True)
                    for q2 in range(NQH):
                        nc.vector.tensor_mul(
                            yts[pr][bass.ds(64 * hh, 64), bass.ds(q2 * QW, QW)],
                            ysbs[(q2, hh)][0:64, :], bc[:, bass.ds(q2 * QW, QW)])
            if kq:
                k_proj(kq.pop(0))

        for g in range(NU * KT):
            u, kk = g // KT, g % KT
            pr, qh = units[u]
            mkt = pm.tile([128, QW], F16, tag="mk", name="mk")
            # issued from the otherwise-idle GpSimd stream so mask prefetch
            # is not serialized behind the sync stream's pool-release waits
            nc.gpsimd.dma_start(
                mkt[:], io["maskT"][bass.ts(kk, 128), bass.ds(qh * QW, QW)])
            s = ps_s.tile([128, 2 * QW], F32, tag="s", name="s")
            for hh in range(2):
                nc.tensor.matmul(
                    s[:, bass.ds(hh * QW, QW)],
                    kt[pr][bass.ds(64 * hh, 64), bass.ts(kk, 128)],
                    qt[pr][bass.ds(64 * hh, 64), bass.ds(qh * QW, QW)],
                    start=True, stop=True,
                    tile_position=(64 * hh, 0))
            e16 = pe.tile([128, 2 * QW], F16, tag="E", name="e16")
            nc.scalar.activation(e16[:], s[:], EXP, scale=scale)
            pend.append((u, kk, e16, mkt))
            if len(pend) > LAG:
                mul_and_av(*pend.pop(0))
        for args in pend:
            mul_and_av(*args)

        if "dbg" in io:
            for e in range(CT):
                nc.sync.dma_start(io["dbg_qt"][bass.ts(e, 128), :], qt[e][:])
                nc.sync.dma_start(io["dbg_kt"][bass.ts(e, 128), :], kt[e][:])
                nc.sync.dma_start(io["dbg_yt"][bass.ts(e, 128), :], yts[e][:])
            for t in range(KT):
                nc.sync.dma_start(
                    io["dbg_v"][:, :].rearrange("p (a b) -> p a b", a=KT)[:, t, :],
                    v[:, t, :, :].rearrange("p a b -> p (a b)"))

        # out[q, :] = sum_e Y^T[e, q] * WuT[e, :] + bu
        pso_tag = "av" if QW * 4 == EMB * 4 else "pso"
        for qi in range(QC // 128):
            pso = ps_av.tile([128, EMB], F32, tag=pso_tag, name="pso")
            for e in range(CT):
                nc.tensor.matmul(pso[:], yts[e][:, bass.ts(qi, 128)], wu[e][:],
                                 start=(e == 0), stop=(e == CT - 1))
            osb = po.tile([128, EMB], F32, tag="o", name="osb")
            nc.vector.tensor_add(osb[:], pso[:], bub[:])
            for hf in range(4):
                eng = nc.sync if hf % 2 == 0 else nc.gpsimd
                eng.dma_start(
                    io["out"][bass.ds(qi * 128 + hf * 32, 32), :],
                    osb[bass.ds(hf * 32, 32), :])


def build(cfg, num_devices=8, dbg=False):
    T, QC = cfg["T"], cfg["QC"]
    nc = bacc.Bacc("TRN2", target_bir_lowering=False, debug=False,
                   num_devices=num_devices)
    io = {
        "x1T": nc.dram_tensor("x1T", [EMB, QC], F16, kind="ExternalInput").ap(),
        "x2T": nc.dram_tensor("x2T", [EMB, T], F16, kind="ExternalInput").ap(),
        "maskT": nc.dram_tensor("maskT", [T, QC], F16, kind="ExternalInput").ap(),
        "wqT": nc.dram_tensor("wqT", [EMB, EMB], F16, kind="ExternalInput").ap(),
        "wkT": nc.dram_tensor("wkT", [EMB, EMB], F16, kind="ExternalInput").ap(),
        "wvT": nc.dram_tensor("wvT", [EMB, EMB], F16, kind="ExternalInput").ap(),
        "wuT": nc.dram_tensor("wuT", [EMB, EMB], F16, kind="ExternalInput").ap(),
        "bqr": nc.dram_tensor("bqr", [128, CT], F32, kind="ExternalInput").ap(),
        "bkr": nc.dram_tensor("bkr", [128, CT], F32, kind="ExternalInput").ap(),
        "bub": nc.dram_tensor("bub", [128, EMB], F32, kind="ExternalInput").ap(),
        "out": nc.dram_tensor("out", [QC, EMB], F32, kind="ExternalOutput").ap(),
    }
    if dbg:
        KT = T // 128
        io["dbg"] = True
        io["dbg_qt"] = nc.dram_tensor("dbg_qt", [EMB, QC], F16, kind="ExternalOutput").ap()
        io["dbg_kt"] = nc.dram_tensor("dbg_kt", [EMB, T], F16, kind="ExternalOutput").ap()
        io["dbg_yt"] = nc.dram_tensor("dbg_yt", [EMB, QC], F16, kind="ExternalOutput").ap()
        io["dbg_v"] = nc.dram_tensor("dbg_v", [128, KT * H * 66], F16, kind="ExternalOutput").ap()
    with tile.TileContext(nc) as tc:
        with ExitStack() as ctx:
            attention_body(ctx, tc, io, cfg)
    nc.compile()
    return nc


def host_prep(x1, x2, mask, Wq, bq, Wk, bk, Wv, bv, Wu, bu, cfg):
    """Build the 8 per-core input maps from full inputs."""
    T, QC = cfg["T"], cfg["QC"]
    shared = {
        "wqT": np.ascontiguousarray(Wq.T).astype(np.float16),
        "wkT": np.ascontiguousarray(Wk.T).astype(np.float16),
        "wvT": np.ascontiguousarray(Wv.T).astype(np.float16),
        "wuT": np.ascontiguousarray(Wu.T).astype(np.float16),
        "bqr": np.ascontiguousarray(bq.reshape(CT, 128).T).astype(np.float32),
        "bkr": np.ascontiguousarray(bk.reshape(CT, 128).T).astype(np.float32),
        # bv folded into the output bias: out = y0@Wu.T + (bu + Wu@bv)
        "bub": np.ascontiguousarray(
            np.broadcast_to(bu + Wu @ bv, (128, EMB))).astype(np.float32),
    }
    x2T = [x2[b].T.astype(np.float16) for b in range(x1.shape[0])]
    in_maps = []
    n_cores = (x1.shape[0] * x1.shape[1]) // QC
    per_b = x1.shape[1] // QC
    for c in range(n_cores):
        b, q0 = c // per_b, (c % per_b) * QC
        in_maps.append(dict(
            shared,
            x1T=x1[b, q0:q0 + QC, :].T.astype(np.float16),
            x2T=x2T[b],
            maskT=mask[b, q0:q0 + QC, :].T.astype(np.float16),
        ))
    return in_maps


_NC_CACHE = {}


def kernel(x1, x2, mask, Wq, bq, Wk, bk, Wv, bv, Wu, bu):
    cfg = FULL_CFG
    B, TQ, _ = x1.shape
    in_maps = host_prep(np.asarray(x1, np.float32), np.asarray(x2, np.float32),
                        np.asarray(mask), np.asarray(Wq, np.float32),
                        np.asarray(bq, np.float32), np.asarray(Wk, np.float32),
                        np.asarray(bk, np.float32), np.asarray(Wv, np.float32),
                        np.asarray(bv, np.float32), np.asarray(Wu, np.float32),
                        np.asarray(bu, np.float32), cfg)
    key = (cfg["T"], cfg["QC"])
    if key not in _NC_CACHE:
        _NC_CACHE[key] = build(cfg)
    nc = _NC_CACHE[key]
    res = run_bass_kernel_spmd(nc, in_maps, core_ids=list(range(8)),
                               trace=bool(os.environ.get("KERNEL_TRACE")))
    if os.environ.get("KERNEL_TRACE"):
        kernel.last_exec_ns = res.exec_time_ns
        kernel.last_results = res
    out = np.empty((B, TQ, EMB), np.float32)
    per_b = TQ // cfg["QC"]
    for c in range(8):
        b, q0 = c // per_b, (c % per_b) * cfg["QC"]
        out[b, q0:q0 + cfg["QC"], :] = res.results[c]["out"]
    return out


# revision 28
# speedup vs baseline: 1.2637x; 1.0204x over previous
"""Multi-head cross attention on 8 trn2 NeuronCores.

Problem: B=2, T=4096, EMB=512, H=8 heads (head dim 64), fp32 I/O.
  q = x1 @ Wq.T + bq ; k,v from x2 ; S = q k^T / sqrt(512) ;
  softmax over keys with -1e10 masking ; out = (A v) @ Wu.T + bu.

Sharding: core c handles batch b = c//4 and query rows
[1024*(c%4), 1024*(c%4+1)).  Each core computes K,V for its batch in
full (4-way duplication), its own Q chunk, attention, and out-proj.

Device-side design (v2) — the kernel is ACT(exp)-bound, so everything
is organized to keep the Activation engine streaming EXPs gaplessly:
  - Scores computed TRANSPOSED, S^T[key, query] (head dim on PE
    partitions), from host-pre-transposed fp16 inputs.  2 heads packed
    per scores pass via tile_position row-tiling.
  - Queries processed in two 512-wide halves so each (head-pair, kk)
    score tile is [128, 1024] = 2 PSUM banks; tag-rotated double
    buffering (4 banks) lets EXP(kk+1) run while scores(kk+2) fill.
  - AV accumulates per head in [65, QW] PSUM (ones column in V gives
    softmax denominators); 2 heads x 1 bank, rotated (2 banks).
  - K projection for e-tiles 1-3 trickles through a single [128,1024]
    PSUM tag (2 banks) DURING attention (DVE bias adds); e-tile 0, Q
    and V are computed up front with ACT/GpSimd bias adds (ACT is idle
    pre-attention; Identity and Exp share an activation table so no
    table reloads).  x2^T is DMA'd column-block-first so K/V proj
    start before the transfer completes.
  - Normalization: r rows copied out via GpSimd, reciprocal on DVE,
    K=1 ones-matmul broadcast, final scale on DVE; the broadcast tile
    reuses the K-proj PSUM tag, out-proj PSUM reuses the av tag, so
    total PSUM = 4+2+2 = 8 banks with no pool barriers inside the
    attention phase.
"""
import math
import os
from contextlib import ExitStack

import numpy as np

import concourse.bass as bass
import concourse.bacc as bacc
import concourse.tile as tile
import concourse.mybir as mybir
from concourse.bass_utils import run_bass_kernel_spmd

F16 = mybir.dt.float16
F32 = mybir.dt.float32
EXP = mybir.ActivationFunctionType.Exp

EMB, H, D, CT = 512, 8, 64, 4  # emb, heads, head dim, emb/128

FULL_CFG = dict(T=4096, QC=1024)  # keys per batch, query rows per core
MINI_CFG = dict(T=512, QC=256)


def attention_body(ctx, tc, io, cfg):
    nc = tc.nc
    T, QC = cfg["T"], cfg["QC"]
    KT = T // 128              # key tiles
    NQH = 2 if QC >= 1024 else 1
    QW = QC // NQH             # query-half width (512 full config)
    NCH = max(1, QC // 512)    # 512-wide matmul chunks per QC
    KG = max(1, T // QC)       # K-proj groups of QC keys per e-tile
    scale = 1.0 / math.sqrt(EMB)

    pw = ctx.enter_context(tc.tile_pool(name="w", bufs=1))
    pk = ctx.enter_context(tc.tile_pool(name="kt", bufs=1))
    pv = ctx.enter_context(tc.tile_pool(name="v", bufs=1))
    pq = ctx.enter_context(tc.tile_pool(name="qt", bufs=1))
    px = ctx.enter_context(tc.tile_pool(name="x", bufs=1))

    # persistent weights / biases / constants; DMA order = need order
    bqr = pw.tile([128, CT], F32, tag="bqr", name="bqr")
    bkr = pw.tile([128, CT], F32, tag="bkr", name="bkr")
    nc.sync.dma_start(bqr[:], io["bqr"][:, :])
    nc.sync.dma_start(bkr[:], io["bkr"][:, :])
    wq = [pw.tile([128, EMB], F16, tag=f"wq{i}", name=f"wq{i}") for i in range(CT)]
    wk = [pw.tile([128, EMB], F16, tag=f"wk{i}", name=f"wk{i}") for i in range(CT)]
    wv = [pw.tile([128, EMB], F16, tag=f"wv{i}", name=f"wv{i}") for i in range(CT)]
    wu = [pw.tile([128, EMB], F16, tag=f"wu{i}", name=f"wu{i}") for i in range(CT)]
    for i in range(CT):
        nc.sync.dma_start(wq[i][:], io["wqT"][bass.ts(i, 128), :])
        nc.sync.dma_start(wk[i][:], io["wkT"][bass.ts(i, 128), :])
        nc.sync.dma_start(wv[i][:], io["wvT"][bass.ts(i, 128), :])
    x1t = [px.tile([128, QC], F16, tag=f"x1t{i}", name=f"x1t{i}") for i in range(CT)]
    for i in range(CT):
        nc.sync.dma_start(x1t[i][:], io["x1T"][bass.ts(i, 128), :])
    # x2^T lands column-block-first so K/V proj can chase the transfer
    x2t = [px.tile([128, T], F16, tag=f"x2t{i}", name=f"x2t{i}") for i in range(CT)]
    XJ = max(1, T // 1024)  # 2 KiB-per-partition chunks for DMA efficiency
    for j in range(XJ):
        for i in range(CT):
            nc.sync.dma_start(x2t[i][:, bass.ts(j, T // XJ)],
                              io["x2T"][bass.ts(i, 128), bass.ts(j, T // XJ)])
    bub = pw.tile([128, EMB], F32, tag="bub", name="bub")
    nc.sync.dma_start(bub[:], io["bub"][:, :])
    for i in range(CT):
        nc.sync.dma_start(wu[i][:], io["wuT"][bass.ts(i, 128), :])
    ones = pw.tile([1, D], F16, tag="ones", name="ones")
    nc.vector.memset(ones[:], 1.0)

    # persistent K^T [emb, T], V per key-tile [key, head, 65(+pad)] (one
    # tile per kk so the 32 projection drains are independent writers),
    # Q^T [emb, QC]
    kt = [pk.tile([128, T], F16, tag=f"kt{i}", name=f"kt{i}") for i in range(CT)]
    v = [pv.tile([128, H, 66], F16, tag=f"v{t}", name=f"v{t}") for t in range(KT)]
    for t in range(KT):
        nc.vector.memset(v[t][:, :, 64:65], 1.0)
    qt = [pq.tile([128, QC], F16, tag=f"qt{i}", name=f"qt{i}") for i in range(CT)]

    # ---- pre-attention projections: Q, then per-x2-chunk K e0 + V ----
    # bv is folded into bub on the host (out = y0@Wu.T + (bu + Wu@bv)),
    # so V just drains PSUM -> fp16 interleaved, split across ACT + DVE.
    # Emission follows x2^T column-arrival order so the in-order PE
    # stream is never parked behind not-yet-landed data.
    with tc.tile_pool(name="pp0", bufs=1, space="PSUM") as pp0:
        for e in range(CT):
            ps = pp0.tile([128, QC], F32, tag="qp", name=f"qps{e}", bufs=2)
            for t in range(NCH):
                for c in range(CT):
                    nc.tensor.matmul(ps[:, bass.ts(t, QC // NCH)],
                                     wq[c][:, bass.ts(e, 128)],
                                     x1t[c][:, bass.ts(t, QC // NCH)],
                                     start=(c == 0), stop=(c == CT - 1))
            nc.scalar.add(qt[e][:], ps[:], bqr[:, e:e + 1])
        for g in range(KG):
            ps = pp0.tile([128, QC], F32, tag="qp", name=f"kps0{g}", bufs=2)
            for t in range(NCH):
                for c in range(CT):
                    nc.tensor.matmul(
                        ps[:, bass.ts(t, QC // NCH)],
                        wk[c][:, 0:128],
                        x2t[c][:, bass.ds(g * QC + t * (QC // NCH), QC // NCH)],
                        start=(c == 0), stop=(c == CT - 1))
            nc.scalar.add(kt[0][:, bass.ts(g, QC)], ps[:], bkr[:, 0:1])
            for t in range(g * KT // KG, (g + 1) * KT // KG):
                ps = pp0.tile([128, EMB], F32, tag="vp", name=f"vps{t}", bufs=4)
                for c in range(CT):
                    nc.tensor.matmul(ps[:], x2t[c][:, bass.ts(t, 128)],
                                     wv[c][:], start=(c == 0), stop=(c == CT - 1))
                vdst = v[t][:, :, 0:64]
                psr = ps[:].rearrange("p (h d) -> p h d", h=H)
                if t % 2 == 0:
                    nc.scalar.copy(vdst, psr)
                else:
                    nc.vector.tensor_copy(vdst, psr)

    # ---- attention -----------------------------------------------------
    with tc.tile_pool(name="ps_s", bufs=2, space="PSUM") as ps_s, \
         tc.tile_pool(name="ps_av", bufs=2, space="PSUM") as ps_av, \
         tc.tile_pool(name="ps_kp", bufs=1, space="PSUM") as ps_kp, \
         tc.tile_pool(name="pe", bufs=5) as pe, \
         tc.tile_pool(name="ppp", bufs=3) as ppp, \
         tc.tile_pool(name="pm", bufs=14) as pm, \
         tc.tile_pool(name="pys", bufs=6) as pys, \
         tc.tile_pool(name="prr", bufs=2) as prr, \
         tc.tile_pool(name="po", bufs=2) as po:
        yts = [py_tile for py_tile in
               (pq.tile([128, QC], F16, tag=f"yt{e}", name=f"yt{e}")
                for e in range(CT))]

        def k_proj(e):
            # trickles through 1-buffer PSUM tag during attention (DVE adds)
            for g in range(KG):
                ps = ps_kp.tile([128, QC], F32, tag="kp", name=f"kps{e}{g}")
                for t in range(NCH):
                    for c in range(CT):
                        nc.tensor.matmul(
                            ps[:, bass.ts(t, QC // NCH)],
                            wk[c][:, bass.ts(e, 128)],
                            x2t[c][:, bass.ds(g * QC + t * (QC // NCH), QC // NCH)],
                            start=(c == 0), stop=(c == CT - 1))
                nc.vector.tensor_scalar_add(kt[e][:, bass.ts(g, QC)], ps[:],
                                            bkr[:, e:e + 1])

        kq = [1, 2, 3]  # pending K-proj e-tiles, emitted between units
        LAG = 2         # mul/AV trail scores/EXP by LAG steps (program order)
        units = [(pr, qh) for pr in range(CT) for qh in range(NQH)]
        NU = len(units)
        avs = {}        # unit -> [av_h0, av_h1]
        ysbs, rrs = {}, {}
        pend = []       # (unit, kk, e16, mkt) awaiting mul+AV

        def mul_and_av(u, kk, e16, mkt):
            pr, qh = units[u]
            if kk == 0:
                avs[u] = [ps_av.tile([65, QW], F32, tag="av", name=f"av{hh}")
                          for hh in range(2)]
            pt = ppp.tile([128, 2 * QW], F16, tag="P", name="pt")
            nc.vector.tensor_mul(
                pt[:].rearrange("p (h q) -> p h q", h=2),
                e16[:].rearrange("p (h q) -> p h q", h=2),
                mkt[:].unsqueeze(1).broadcast_to([128, 2, QW]))
            for hh in range(2):
                nc.tensor.matmul(
                    avs[u][hh][:], v[kk][:, 2 * pr + hh, 0:65],
                    pt[:, bass.ds(hh * QW, QW)],
                    start=(kk == 0), stop=(kk == KT - 1))
            if kk == KT - 1:
                finish_unit(u)

        def finish_unit(u):
            # drain r + Y^T out of PSUM so av buffers recycle fast; the
            # last unit's drains go to ACT (idle once EXPs are done) to
            # shorten the serial DVE tail
            pr, qh = units[u]
            for hh in range(2):
                ysb = pys.tile([65, QW], F32, tag="ys", name="ysb")
                if u == NU - 1:
                    nc.scalar.copy(ysb[:], avs[u][hh][:])
                else:
                    nc.vector.tensor_copy(ysb[:], avs[u][hh][:])
                r0 = prr.tile([1, QW], F32, tag="r0", name="r0")
                nc.vector.tensor_copy(r0[:], ysb[64:65, :])
                rr32 = prr.tile([1, QW], F32, tag="rr32", name="rr32")
                nc.vector.reciprocal_approx_fast(rr32[:], r0[:])
                rr = prr.tile([1, QW], F16, tag="rr", name="rr", bufs=4)
                with nc.allow_low_precision(reason="fp16 recip copy ok"):
                    nc.vector.tensor_copy(rr[:], rr32[:])
                ysbs[(qh, hh)] = ysb
                rrs[(qh, hh)] = rr
            if qh == NQH - 1:
                # normalize: Y^T_h / r_h via K=1 ones-matmul broadcast.
                # Deferred a few steps into the next unit so the bc
                # matmuls (which wait on the DVE reciprocal chain) don't
                # park the in-order PE stream ahead of its next scores.
                snap = dict(ysbs=dict(ysbs), rrs=dict(rrs), pr=pr)

                def normalize(sn=snap):
                    for hh in range(2):
                        bc = ps_kp.tile([64, QC], F32, tag="kp", name="bc")
                        for q2 in range(NQH):
                            nc.tensor.matmul(bc[:, bass.ds(q2 * QW, QW)],
                                             ones[:], sn["rrs"][(q2, hh)][:],
                                             start=True, stop=True)
                        for q2 in range(NQH):
                            nc.vector.tensor_mul(
                                yts[sn["pr"]][bass.ds(64 * hh, 64),
                                              bass.ds(q2 * QW, QW)],
                                sn["ysbs"][(q2, hh)][0:64, :],
                                bc[:, bass.ds(q2 * QW, QW)])
                deferred.append(normalize)
            if kq:
                e = kq.pop(0)
                deferred.append(lambda e=e: k_proj(e))

        deferred = []
        for g in range(NU * KT + LAG):
            if g >= NU * KT:
                mul_and_av(*pend.pop(0))
                continue
            u, kk = g // KT, g % KT
            pr, qh = units[u]
            if kk == 4 and deferred:
                deferred.pop(0)()
            if kk == 12 and deferred:
                deferred.pop(0)()
            mkt = pm.tile([128, QW], F16, tag="mk", name="mk")
            # issued from the otherwise-idle GpSimd stream so mask prefetch
            # is not serialized behind the sync stream's pool-release waits
            nc.gpsimd.dma_start(
                mkt[:], io["maskT"][bass.ts(kk, 128), bass.ds(qh * QW, QW)])
            s = ps_s.tile([128, 2 * QW], F32, tag="s", name="s")
            for hh in range(2):
                nc.tensor.matmul(
                    s[:, bass.ds(hh * QW, QW)],
                    kt[pr][bass.ds(64 * hh, 64), bass.ts(kk, 128)],
                    qt[pr][bass.ds(64 * hh, 64), bass.ds(qh * QW, QW)],
                    start=True, stop=True,
                    tile_position=(64 * hh, 0))
            e16 = pe.tile([128, 2 * QW], F16, tag="E", name="e16")
            nc.scalar.activation(e16[:], s[:], EXP, scale=scale)
            pend.append((u, kk, e16, mkt))
            if len(pend) > LAG:
                mul_and_av(*pend.pop(0))
        for fn in deferred:
            fn()

        if "dbg" in io:
            for e in range(CT):
                nc.sync.dma_start(io["dbg_qt"][bass.ts(e, 128), :], qt[e][:])
                nc.sync.dma_start(io["dbg_kt"][bass.ts(e, 128), :], kt[e][:])
                nc.sync.dma_start(io["dbg_yt"][bass.ts(e, 128), :], yts[e][:])
            for t in range(KT):
                nc.sync.dma_start(
                    io["dbg_v"][:, :].rearrange("p (a b) -> p a b", a=KT)[:, t, :],
                    v[t][:, :, :].rearrange("p a b -> p (a b)"))

        # out[q, :] = sum_e Y^T[e, q] * WuT[e, :] + bu
        pso_tag = "av" if QW * 4 == EMB * 4 else "pso"
        for qi in range(QC // 128):
            pso = ps_av.tile([128, EMB], F32, tag=pso_tag, name="pso")
            for e in range(CT):
                nc.tensor.matmul(pso[:], yts[e][:, bass.ts(qi, 128)], wu[e][:],
                                 start=(e == 0), stop=(e == CT - 1))
            osb = po.tile([128, EMB], F32, tag="o", name="osb")
            nc.vector.tensor_add(osb[:], pso[:], bub[:])
            for hf in range(4):
                eng = nc.sync if hf % 2 == 0 else nc.gpsimd
                eng.dma_start(
                    io["out"][bass.ds(qi * 128 + hf * 32, 32), :],
                    osb[bass.ds(hf * 32, 32), :])


def build(cfg, num_devices=8, dbg=False):
    T, QC = cfg["T"], cfg["QC"]
    nc = bacc.Bacc("TRN2", target_bir_lowering=False, debug=False,
                   num_devices=num_devices)
    io = {
        "x1T": nc.dram_tensor("x1T", [EMB, QC], F16, kind="ExternalInput").ap(),
        "x2T": nc.dram_tensor("x2T", [EMB, T], F16, kind="ExternalInput").ap(),
        "maskT": nc.dram_tensor("maskT", [T, QC], F16, kind="ExternalInput").ap(),
        "wqT": nc.dram_tensor("wqT", [EMB, EMB], F16, kind="ExternalInput").ap(),
        "wkT": nc.dram_tensor("wkT", [EMB, EMB], F16, kind="ExternalInput").ap(),
        "wvT": nc.dram_tensor("wvT", [EMB, EMB], F16, kind="ExternalInput").ap(),
        "wuT": nc.dram_tensor("wuT", [EMB, EMB], F16, kind="ExternalInput").ap(),
        "bqr": nc.dram_tensor("bqr", [128, CT], F32, kind="ExternalInput").ap(),
        "bkr": nc.dram_tensor("bkr", [128, CT], F32, kind="ExternalInput").ap(),
        "bub": nc.dram_tensor("bub", [128, EMB], F32, kind="ExternalInput").ap(),
        "out": nc.dram_tensor("out", [QC, EMB], F32, kind="ExternalOutput").ap(),
    }
    if dbg:
        KT = T // 128
        io["dbg"] = True
        io["dbg_qt"] = nc.dram_tensor("dbg_qt", [EMB, QC], F16, kind="ExternalOutput").ap()
        io["dbg_kt"] = nc.dram_tensor("dbg_kt", [EMB, T], F16, kind="ExternalOutput").ap()
        io["dbg_yt"] = nc.dram_tensor("dbg_yt", [EMB, QC], F16, kind="ExternalOutput").ap()
        io["dbg_v"] = nc.dram_tensor("dbg_v", [128, KT * H * 66], F16, kind="ExternalOutput").ap()
    with tile.TileContext(nc) as tc:
        with ExitStack() as ctx:
            attention_body(ctx, tc, io, cfg)
    nc.compile()
    return nc


def host_prep(x1, x2, mask, Wq, bq, Wk, bk, Wv, bv, Wu, bu, cfg):
    """Build the 8 per-core input maps from full inputs."""
    T, QC = cfg["T"], cfg["QC"]
    shared = {
        "wqT": np.ascontiguousarray(Wq.T).astype(np.float16),
        "wkT": np.ascontiguousarray(Wk.T).astype(np.float16),
        "wvT": np.ascontiguousarray(Wv.T).astype(np.float16),
        "wuT": np.ascontiguousarray(Wu.T).astype(np.float16),
        "bqr": np.ascontiguousarray(bq.reshape(CT, 128).T).astype(np.float32),
        "bkr": np.ascontiguousarray(bk.reshape(CT, 128).T).astype(np.float32),
        # bv folded into the output bias: out = y0@Wu.T + (bu + Wu@bv)
        "bub": np.ascontiguousarray(
            np.broadcast_to(bu + Wu @ bv, (128, EMB))).astype(np.float32),
    }
    x2T = [x2[b].T.astype(np.float16) for b in range(x1.shape[0])]
    in_maps = []
    n_cores = (x1.shape[0] * x1.shape[1]) // QC
    per_b = x1.shape[1] // QC
    for c in range(n_cores):
        b, q0 = c // per_b, (c % per_b) * QC
        in_maps.append(dict(
            shared,
            x1T=x1[b, q0:q0 + QC, :].T.astype(np.float16),
            x2T=x2T[b],
            maskT=mask[b, q0:q0 + QC, :].T.astype(np.float16),
        ))
    return in_maps


_NC_CACHE = {}


def kernel(x1, x2, mask, Wq, bq, Wk, bk, Wv, bv, Wu, bu):
    cfg = FULL_CFG
    B, TQ, _ = x1.shape
    in_maps = host_prep(np.asarray(x1, np.float32), np.asarray(x2, np.float32),
                        np.asarray(mask), np.asarray(Wq, np.float32),
                        np.asarray(bq, np.float32), np.asarray(Wk, np.float32),
                        np.asarray(bk, np.float32), np.asarray(Wv, np.float32),
                        np.asarray(bv, np.float32), np.asarray(Wu, np.float32),
                        np.asarray(bu, np.float32), cfg)
    key = (cfg["T"], cfg["QC"])
    if key not in _NC_CACHE:
        _NC_CACHE[key] = build(cfg)
    nc = _NC_CACHE[key]
    res = run_bass_kernel_spmd(nc, in_maps, core_ids=list(range(8)),
                               trace=bool(os.environ.get("KERNEL_TRACE")))
    if os.environ.get("KERNEL_TRACE"):
        kernel.last_exec_ns = res.exec_time_ns
        kernel.last_results = res
    out = np.empty((B, TQ, EMB), np.float32)
    per_b = TQ // cfg["QC"]
    for c in range(8):
        b, q0 = c // per_b, (c % per_b) * cfg["QC"]
        out[b, q0:q0 + cfg["QC"], :] = res.results[c]["out"]
    return out
